# revision 12
# baseline (speedup 1.0000x reference)
"""Trainium2 Bass kernel for nn_DN_21758304321874 (DN / vq_codebook train loop).

Strategy
--------
The reference runs 10 sequential "train" steps, each of which L2-normalizes the
full x2y_w [16384, 4096], z2y_w [16384, 100], y2z_w [100, 16384] matrices and
does a full [Y,X] matvec — but each step only *modifies one row* of each
matrix.  Mathematically the entire loop collapses to ONE streaming pass over
the weights computing per-row quantities, followed by a tiny O(Y) sequential
recurrence:

  device (sharded Y across 8 cores; memory-bound pass, reads every weight once):
      d[i]  = dot(x2y_w[i], xv)          n2[i] = sum(x2y_w[i]^2)
      c[i]  = dot(z2y_w[i], zh)          m2[i] = sum(z2y_w[i]^2)
      q2[k] = partial row sums of y2z_w[k]^2   (column-sharded)

  host (tiny): 10-step winner/threshold recurrence on the [16384] vectors,
      analytic EMA+renorm updates of the (<=10) winner rows, final argmax and
      assembly of the [1, 100] output + activated count.

Sharding: Y dimension (rows of x2y/z2y, columns of y2z) split across 8 cores,
per the sharding hint.  No collectives needed; host combines 8 small outputs.
"""

import numpy as np

import concourse.bacc as bacc
import concourse.bass as bass
import concourse.tile as tile
from concourse import mybir
from concourse.bass_utils import run_bass_kernel_spmd

EPS = 1e-12

N_CORES = 8
Y, X, Z = 16384, 4096, 100
RPC = Y // N_CORES          # rows per core = 2048
NT = RPC // 128             # row tiles per core = 16

_PROGRAM = None
LAST_RESULTS = None  # BassKernelResults of the most recent device run (for test.py)


def _emit_body(nc, tc, x2y, z2y, y2z, xv, zh, out, reps=1):
    """Emit the streaming pass.  reps>1 re-reads x2y (timing variants only)."""
    f32 = mybir.dt.float32
    with (
        tc.tile_pool(name="consts", bufs=1) as consts,
        tc.tile_pool(name="work", bufs=3) as work,
        tc.tile_pool(name="prods", bufs=2) as prods,
        tc.tile_pool(name="sqs", bufs=2) as sqs,
        tc.tile_pool(name="outp", bufs=1) as outp,
    ):
        ostage = outp.tile([128, 65], f32)
        nc.vector.memset(ostage, 0.0)

        xvb = consts.tile([128, X], f32)
        nc.gpsimd.dma_start(out=xvb, in_=xv[:].partition_broadcast(128))

        # ---- z2y pass: c = z2y @ zh, m2 = row sums of z2y^2 ----
        # contiguous reshape: partition p holds rows p*16 .. p*16+15.
        zhb = consts.tile([128, NT * Z], f32)
        zh_rep = bass.AP(tensor=zh[:].tensor, offset=0,
                         ap=[[0, 128], [0, NT], [1, Z]])
        nc.gpsimd.dma_start(out=zhb, in_=zh_rep)
        zw = work.tile([128, NT * Z], f32, tag="zw")
        nc.sync.dma_start(out=zw,
                          in_=z2y[:].rearrange("(p j) m -> p (j m)", p=128))
        zprod = prods.tile([128, NT * Z], f32, tag="zprod")
        nc.vector.tensor_mul(out=zprod, in0=zw, in1=zhb)
        nc.vector.reduce_sum(out=ostage[:, 32:48],
                             in_=zprod.rearrange("p (j m) -> p j m", m=Z),
                             axis=mybir.AxisListType.X)
        nc.vector.tensor_mul(out=zprod, in0=zw, in1=zw)
        nc.vector.reduce_sum(out=ostage[:, 48:64],
                             in_=zprod.rearrange("p (j m) -> p j m", m=Z),
                             axis=mybir.AxisListType.X)

        # ---- y2z pass: partial row sums of y2z^2 (ACT square + row-accum) ----
        yw = work.tile([Z, RPC], f32, tag="yw")
        nc.sync.dma_start(out=yw, in_=y2z[:])
        ysq = sqs.tile([Z, RPC], f32, tag="ysq")
        nc.scalar.activation(out=ysq, in_=yw,
                             func=mybir.ActivationFunctionType.Square,
                             accum_out=ostage[:Z, 64:65])

        # ---- x2y pass: d = x2y @ xv (DVE fused mul+reduce),
        #                n2 = row sums of x2y^2 (ACT fused square+reduce) ----
        for rep in range(reps):
            for t in range(NT):
                w = work.tile([128, X], f32, tag="w")
                nc.sync.dma_start(out=w, in_=x2y[t * 128:(t + 1) * 128, :])
                prod = prods.tile([128, X], f32, tag="prod")
                nc.vector.tensor_mul(out=prod, in0=w, in1=xvb)
                nc.vector.reduce_sum(out=ostage[:, t:t + 1], in_=prod,
                                     axis=mybir.AxisListType.X)
                sq = sqs.tile([128, X], f32, tag="sq")
                nc.scalar.activation(out=sq, in_=w,
                                     func=mybir.ActivationFunctionType.Square,
                                     accum_out=ostage[:, 16 + t:17 + t])

        nc.sync.dma_start(out=out[:], in_=ostage)


def _build_program(reps=1):
    """One SPMD bass program; every core runs the same code on its shard."""
    nc = bacc.Bacc()
    f32 = mybir.dt.float32

    x2y = nc.declare_dram_parameter("x2y", [RPC, X], f32, isOutput=False)
    z2y = nc.declare_dram_parameter("z2y", [RPC, Z], f32, isOutput=False)
    y2z = nc.declare_dram_parameter("y2z", [Z, RPC], f32, isOutput=False)
    xv = nc.declare_dram_parameter("xv", [X], f32, isOutput=False)
    zh = nc.declare_dram_parameter("zh", [Z], f32, isOutput=False)
    out = nc.declare_dram_parameter("out", [128, 65], f32, isOutput=True)

    with tile.TileContext(nc) as tc:
        _emit_body(nc, tc, x2y, z2y, y2z, xv, zh, out, reps=reps)

    nc.compile()
    return nc


def _host_finish(d, n2, c, m2, q2, x, z, per_item, y2z_w, age_y, age_z, thr):
    """Tiny sequential recurrence replicating the 10-step train loop."""
    z_idx = int(np.asarray(z).reshape(-1)[0])
    per_item = int(per_item)

    xv = np.asarray(x, np.float32).reshape(-1)
    s = np.float32(np.linalg.norm(xv))
    xv = xv / max(s, np.float32(EPS))
    s2 = np.float32(xv @ xv)

    n = np.sqrt(n2.astype(np.float32))
    m = np.sqrt(m2.astype(np.float32))
    q = np.sqrt(q2.astype(np.float32))

    bu = d / np.maximum(n, EPS)       # dot(unit x2y row, xv)
    td = c / np.maximum(m, EPS)       # unit z2y row value at z_idx
    ypre = np.float32(0.5) * bu + np.float32(0.5) * td

    age_y = np.asarray(age_y, np.float32).copy()
    age_z = np.asarray(age_z, np.float32).copy()
    thr = np.asarray(thr, np.float32).copy()

    # dense evolving row z_idx of y2z_w (unit at each step start)
    r = np.asarray(y2z_w[z_idx], np.float32) / max(q[z_idx], np.float32(EPS))

    for _ in range(per_item):
        i0 = int(np.argmax(ypre))
        r0 = ypre[i0]
        keep = (r0 > thr[i0]) or (age_y[i0] < 1.0)
        unact = (age_y < 1.0).astype(np.float32)
        has_unact = unact.sum() > 0
        alt = int(np.argmax(ypre * unact))
        win = i0 if keep else (alt if has_unact else i0)
        lr = np.float32(1.0) / (age_y[win] + np.float32(1.0))

        # x2y row win: v = (1-lr)*u + lr*xv, with dot(u,xv)=bu[win], |u|=1
        b = bu[win]
        nn = np.sqrt((1 - lr) ** 2 + lr * lr * s2 + 2 * lr * (1 - lr) * b)
        bu[win] = ((1 - lr) * b + lr * s2) / max(nn, np.float32(EPS))

        # z2y row win: v = (1-lr)*u + lr*onehot(z_idx), u[z_idx]=td[win], |u|=1
        t = td[win]
        nz = np.sqrt((1 - lr) ** 2 + lr * lr + 2 * lr * (1 - lr) * t)
        td[win] = ((1 - lr) * t + lr) / max(nz, np.float32(EPS))

        ypre[win] = np.float32(0.5) * bu[win] + np.float32(0.5) * td[win]
        age_y[win] += 1.0

        zlr = np.float32(1.0) / (age_z[z_idx] + np.float32(1.0))
        r = (1 - zlr) * r
        r[win] += zlr
        age_z[z_idx] += 1.0
        thr[win] = lr * r0 + (1 - lr) * thr[win]
        r = r / max(np.float32(np.linalg.norm(r)), np.float32(EPS))

    y_flag = (age_y >= 1.0)
    y_temp = np.where(y_flag, bu, np.float32(0.0))
    jwin = int(np.argmax(y_temp))

    output = np.asarray(y2z_w[:, jwin], np.float32) / np.maximum(q, np.float32(EPS))
    output[z_idx] = r[jwin]
    y_activated_num = np.float32(y_flag.sum())
    return output[None, :].astype(np.float32), y_activated_num


def _get_program():
    global _PROGRAM
    if _PROGRAM is None:
        _PROGRAM = _build_program()
    return _PROGRAM


def _run_device(x2y_w, z2y_w, y2z_w, xv, zh, **spmd_kwargs):
    nc = _get_program()
    in_maps = []
    for cc in range(N_CORES):
        rows = slice(cc * RPC, (cc + 1) * RPC)
        in_maps.append({
            "x2y": np.ascontiguousarray(x2y_w[rows]),
            "z2y": np.ascontiguousarray(z2y_w[rows]),
            "y2z": np.ascontiguousarray(y2z_w[:, rows]),
            "xv": xv,
            "zh": zh,
        })
    res = run_bass_kernel_spmd(nc, in_maps, list(range(N_CORES)), **spmd_kwargs)
    global LAST_RESULTS
    LAST_RESULTS = res

    d = np.empty(Y, np.float32)
    n2 = np.empty(Y, np.float32)
    c = np.empty(Y, np.float32)
    m2 = np.empty(Y, np.float32)
    q2 = np.zeros(Z, np.float32)
    for cc in range(N_CORES):
        o = res.results[cc]["out"]
        rows = slice(cc * RPC, (cc + 1) * RPC)
        d[rows] = o[:, 0:16].T.reshape(-1)     # tile-major: row = t*128 + p
        n2[rows] = o[:, 16:32].T.reshape(-1)
        c[rows] = o[:, 32:48].reshape(-1)      # partition-major: row = p*16 + j
        m2[rows] = o[:, 48:64].reshape(-1)
        q2 += o[:Z, 64]
    return d, n2, c, m2, q2, res


def kernel(x, z, per_item, x2y_w, z2y_w, y2z_w, age_y, age_z, thr,
           _spmd_kwargs=None):
    x2y_w = np.asarray(x2y_w, np.float32)
    z2y_w = np.asarray(z2y_w, np.float32)
    y2z_w = np.asarray(y2z_w, np.float32)
    z_idx = int(np.asarray(z).reshape(-1)[0])

    xv = np.asarray(x, np.float32).reshape(-1)
    xv = xv / max(np.float32(np.linalg.norm(xv)), np.float32(EPS))
    xv = np.ascontiguousarray(xv, np.float32)
    zh = np.zeros(Z, np.float32)
    zh[z_idx] = 1.0

    d, n2, c, m2, q2, _res = _run_device(
        x2y_w, z2y_w, y2z_w, xv, zh, **(_spmd_kwargs or {}))

    return _host_finish(d, n2, c, m2, q2, x, z, per_item, y2z_w,
                        age_y, age_z, thr)
